# revision 1
# baseline (speedup 1.0000x reference)
"""Trainium kernel for nn_Attention_global_63909113364884.

Data-parallel over batch: 8 samples -> 8 NeuronCores. Each core runs the
full per-sample pipeline (qkv 1x1 conv, depthwise 3x3 conv, pixel-unshuffle
tokenization, l2-normed attention with softmax_1, inverse tokenization,
projection), expressed in JAX and compiled for the Neuron cores; inputs are
sharded on the batch dim and the full (8, 48, 256, 256) output is returned.
"""

import jax
import jax.numpy as jnp
import numpy as np
from jax import lax
from functools import partial

B, C, H, W = 8, 48, 256, 256
HEADS = 8
FCT = 16
EPS = 1e-12


def _to_tokens(t):
    b = t.shape[0]
    c = C // HEADS
    h, w = H // FCT, W // FCT
    t = t.reshape(b, HEADS, c, h, FCT, w, FCT)
    t = t.transpose(0, 1, 6, 4, 2, 3, 5)
    return t.reshape(b, HEADS, FCT * FCT, c * h * w)


def _from_tokens(t):
    b = t.shape[0]
    c = C // HEADS
    h, w = H // FCT, W // FCT
    t = t.reshape(b, HEADS, FCT, FCT, c, h, w)
    t = t.transpose(0, 1, 4, 5, 3, 6, 2)
    return t.reshape(b, C, H, W)


def _l2norm(t):
    n = jnp.sqrt(jnp.sum(t * t, axis=-1, keepdims=True))
    return t / jnp.maximum(n, EPS)


def _forward(x, w_qkv, w_dw, w_proj, temperature):
    # x: (1, C, H, W) shard on one core
    qkv = jnp.einsum('bchw,oc->bohw', x, w_qkv)
    qkv = lax.conv_general_dilated(
        qkv, w_dw, window_strides=(1, 1), padding='SAME',
        feature_group_count=3 * C,
        dimension_numbers=('NCHW', 'OIHW', 'NCHW'))
    q, k, v = jnp.split(qkv, 3, axis=1)
    q = _l2norm(_to_tokens(q))
    k = _l2norm(_to_tokens(k))
    v = _to_tokens(v)
    attn = jnp.einsum('bhnd,bhmd->bhnm', q, k) * temperature[None]
    e = jnp.exp(attn)
    attn = e / (jnp.sum(e, axis=-1, keepdims=True) + 1.0)
    out = jnp.einsum('bhnm,bhmd->bhnd', attn, v)
    out = _from_tokens(out)
    return jnp.einsum('bchw,oc->bohw', out, w_proj)


_COMPILED = {}


def _get_fn():
    if 'fn' not in _COMPILED:
        devices = jax.devices()[:8]
        mesh = jax.sharding.Mesh(np.array(devices), ('b',))
        P = jax.sharding.PartitionSpec
        sh_b = jax.sharding.NamedSharding(mesh, P('b'))
        sh_r = jax.sharding.NamedSharding(mesh, P())
        fn = jax.jit(
            _forward,
            in_shardings=(sh_b, sh_r, sh_r, sh_r, sh_r),
            out_shardings=sh_b,
        )
        _COMPILED['fn'] = fn
    return _COMPILED['fn']


def kernel(x, w_qkv, w_dw, w_proj, temperature):
    x = np.asarray(x, dtype=np.float32)
    w_qkv = np.asarray(w_qkv, dtype=np.float32)
    w_dw = np.asarray(w_dw, dtype=np.float32)
    w_proj = np.asarray(w_proj, dtype=np.float32)
    temperature = np.asarray(temperature, dtype=np.float32)
    fn = _get_fn()
    out = fn(x, w_qkv, w_dw, w_proj, temperature)
    return np.asarray(jax.device_get(out), dtype=np.float32)


if __name__ == '__main__':
    rng = np.random.default_rng(0)
    x = rng.standard_normal((B, C, H, W), dtype=np.float32)
    w_qkv = rng.standard_normal((3 * C, C), dtype=np.float32) * 0.05
    w_dw = rng.standard_normal((3 * C, 1, 3, 3), dtype=np.float32) * 0.05
    w_proj = rng.standard_normal((C, C), dtype=np.float32) * 0.05
    temperature = np.ones((HEADS, 1, 1), dtype=np.float32)
    out = kernel(x=x, w_qkv=w_qkv, w_dw=w_dw, w_proj=w_proj,
                 temperature=temperature)
    print(out.shape, out.dtype, float(np.abs(out).max()))


# revision 2
# speedup vs baseline: 1.0203x; 1.0203x over previous
"""Trainium kernel for nn_Attention_global_63909113364884.

Data-parallel over batch: 8 samples -> 8 NeuronCores. Each core runs the
full per-sample pipeline (qkv 1x1 conv, depthwise 3x3 conv, pixel-unshuffle
tokenization, l2-normed attention with softmax_1, inverse tokenization,
projection), expressed in JAX and compiled for the Neuron cores; inputs are
sharded on the batch dim and the full (8, 48, 256, 256) output is returned.
"""

import jax
import jax.numpy as jnp
import numpy as np
from jax import lax
from functools import partial

B, C, H, W = 8, 48, 256, 256
HEADS = 8
FCT = 16
EPS = 1e-12


def _to_tokens(t):
    b = t.shape[0]
    c = C // HEADS
    h, w = H // FCT, W // FCT
    t = t.reshape(b, HEADS, c, h, FCT, w, FCT)
    t = t.transpose(0, 1, 6, 4, 2, 3, 5)
    return t.reshape(b, HEADS, FCT * FCT, c * h * w)


def _from_tokens(t):
    b = t.shape[0]
    c = C // HEADS
    h, w = H // FCT, W // FCT
    t = t.reshape(b, HEADS, FCT, FCT, c, h, w)
    t = t.transpose(0, 1, 4, 5, 3, 6, 2)
    return t.reshape(b, C, H, W)


def _l2norm(t):
    n = jnp.sqrt(jnp.sum(t * t, axis=-1, keepdims=True))
    return t / jnp.maximum(n, EPS)


def _forward(x, w_qkv, w_dw, w_proj, temperature):
    # x: (1, C, H, W) shard on one core. Matmul-heavy ops run with bf16
    # inputs + fp32 accumulation (fp32 matmul is 4x slower on the PE);
    # softmax/normalization stay fp32.
    bf = jnp.bfloat16
    f32 = jnp.float32
    qkv = jnp.einsum('bchw,oc->bohw', x.astype(bf), w_qkv.astype(bf),
                     preferred_element_type=f32)
    qkv = lax.conv_general_dilated(
        qkv.astype(bf), w_dw.astype(bf), window_strides=(1, 1),
        padding='SAME', feature_group_count=3 * C,
        dimension_numbers=('NCHW', 'OIHW', 'NCHW'),
        preferred_element_type=f32)
    q, k, v = jnp.split(qkv, 3, axis=1)
    q = _l2norm(_to_tokens(q))
    k = _l2norm(_to_tokens(k))
    v = _to_tokens(v)
    attn = jnp.einsum('bhnd,bhmd->bhnm', q.astype(bf), k.astype(bf),
                      preferred_element_type=f32) * temperature[None]
    e = jnp.exp(attn)
    attn = e / (jnp.sum(e, axis=-1, keepdims=True) + 1.0)
    out = jnp.einsum('bhnm,bhmd->bhnd', attn.astype(bf), v.astype(bf),
                     preferred_element_type=f32)
    out = _from_tokens(out)
    return jnp.einsum('bchw,oc->bohw', out.astype(bf), w_proj.astype(bf),
                      preferred_element_type=f32)


_COMPILED = {}


def _get_fn():
    if 'fn' not in _COMPILED:
        devices = jax.devices()[:8]
        mesh = jax.sharding.Mesh(np.array(devices), ('b',))
        P = jax.sharding.PartitionSpec
        sh_b = jax.sharding.NamedSharding(mesh, P('b'))
        sh_r = jax.sharding.NamedSharding(mesh, P())
        fn = jax.jit(
            _forward,
            in_shardings=(sh_b, sh_r, sh_r, sh_r, sh_r),
            out_shardings=sh_b,
        )
        _COMPILED['fn'] = fn
    return _COMPILED['fn']


def kernel(x, w_qkv, w_dw, w_proj, temperature):
    x = np.asarray(x, dtype=np.float32)
    w_qkv = np.asarray(w_qkv, dtype=np.float32)
    w_dw = np.asarray(w_dw, dtype=np.float32)
    w_proj = np.asarray(w_proj, dtype=np.float32)
    temperature = np.asarray(temperature, dtype=np.float32)
    fn = _get_fn()
    out = fn(x, w_qkv, w_dw, w_proj, temperature)
    return np.asarray(jax.device_get(out), dtype=np.float32)


if __name__ == '__main__':
    rng = np.random.default_rng(0)
    x = rng.standard_normal((B, C, H, W), dtype=np.float32)
    w_qkv = rng.standard_normal((3 * C, C), dtype=np.float32) * 0.05
    w_dw = rng.standard_normal((3 * C, 1, 3, 3), dtype=np.float32) * 0.05
    w_proj = rng.standard_normal((C, C), dtype=np.float32) * 0.05
    temperature = np.ones((HEADS, 1, 1), dtype=np.float32)
    out = kernel(x=x, w_qkv=w_qkv, w_dw=w_dw, w_proj=w_proj,
                 temperature=temperature)
    print(out.shape, out.dtype, float(np.abs(out).max()))
